# revision 1
# baseline (speedup 1.0000x reference)
"""BertImageSelfAttention Trainium2 kernel.

Shapes (fixed): hidden_states [4, 2048, 1024], 16 heads x 64, text [4, 64, 768].
Sharding: 8 cores = 4 batches x 2 head-groups (8 heads each). Each core computes
its batch's attention context for its 8 heads; host reassembles [4, 2048, 1024].

Per-core device pipeline (all matmuls bf16 with fp32 PSUM accumulation):
  A. pooled text -> dynamic Q/K gates (tiny matmuls + sigmoid)
  B. load x^T (host pre-transposed, bf16) + weight slices
  C. projections: Q^T,K^T [512e, 2048s] (gates+bias fused into PSUM eviction),
     V [2048t, 512e] augmented with a ones column per head (denominator trick)
  D. per head: S^T tiles = K^T.T @ Q^T (keys on partitions), ACT Exp with
     scale=1/8 and per-partition bias=attention_mask (exact softmax mask fold),
     ctx^T[65,512] += Vaug.T @ E^T accumulated over 16 key chunks
     (row 64 = softmax denominator)
  E. PE-transpose ctx^T -> [s, 65], multiply by reciprocal denominator, add bv,
     DMA out [2048, 512] fp32.
"""

import os

import numpy as np
import ml_dtypes

import concourse.bass as bass
import concourse.bacc as bacc
import concourse.tile as tile
from concourse import mybir
from concourse.bass_utils import run_bass_kernel_spmd

P = 128
B, S, DV = 4, 2048, 1024
H, Dh = 16, 64
T, DT = 64, 768
NCORES = 8
E = 512          # head-group width (8 heads x 64)
CC = DV // P     # 8 contraction chunks for projections
ECH = E // P     # 4 e-chunks
DC = DT // P     # 6 text-dim chunks
SC = S // P      # 16 seq chunks of 128
SBL = S // 512   # 4 seq blocks of 512
HPC = 8          # heads per core

FP32 = mybir.dt.float32
BF16 = mybir.dt.bfloat16
AF = mybir.ActivationFunctionType
OP = mybir.AluOpType

BF16_NP = ml_dtypes.bfloat16

_CACHE = {}

# module-level stash of the last BassKernelResults (for test.py introspection)
last_results = None


def _emit(tc, aps):
    nc = tc.nc
    xT = aps["xT"].rearrange("(c p) s -> p c s", p=P)          # [128, 8, 2048]
    wq = aps["wq"].rearrange("(c p) e -> p c e", p=P)          # [128, 8, 512]
    wk = aps["wk"].rearrange("(c p) e -> p c e", p=P)
    wv = aps["wv"].rearrange("(c p) e -> p c e", p=P)
    wdq = aps["wdq"].rearrange("(c p) e -> p c e", p=P)        # [128, 6, 512]
    wdk = aps["wdk"].rearrange("(c p) e -> p c e", p=P)
    txt = aps["txt"]                                           # [64, 768] bf16
    tmask = aps["tmask"]                                       # [64, 1] bf16
    amask = aps["amask"].rearrange("(c p) -> p c", p=P)        # [128, 16]
    bq = aps["bq"].rearrange("(c p) -> p c", p=P)              # [128, 4]
    bk = aps["bk"].rearrange("(c p) -> p c", p=P)
    bdq = aps["bdq"].rearrange("(c p) -> p c", p=P)
    bdk = aps["bdk"].rearrange("(c p) -> p c", p=P)
    bv = aps["bv"]                                             # [512]
    out = aps["out"]                                           # [8, 64, 2048] f32

    from contextlib import ExitStack

    with ExitStack() as ctx:
        wpool = ctx.enter_context(tc.tile_pool(name="wpool", bufs=1))
        xpool = ctx.enter_context(tc.tile_pool(name="xpool", bufs=1))
        qkpool = ctx.enter_context(tc.tile_pool(name="qkpool", bufs=1))
        vpool = ctx.enter_context(tc.tile_pool(name="vpool", bufs=1))
        etp = ctx.enter_context(tc.tile_pool(name="etp", bufs=5))
        rbp = ctx.enter_context(tc.tile_pool(name="rbp", bufs=3))
        outp = ctx.enter_context(tc.tile_pool(name="outp", bufs=4))
        smallp = ctx.enter_context(tc.tile_pool(name="smallp", bufs=1))
        rcp = ctx.enter_context(tc.tile_pool(name="rcp", bufs=4))
        # PSUM: 8 banks = scp 2 x [128,1024] (2 banks each) + accp 4 x 1 bank
        accp = ctx.enter_context(tc.tile_pool(name="accp", bufs=4, space="PSUM"))
        scp = ctx.enter_context(tc.tile_pool(name="scp", bufs=2, space="PSUM"))

        # text tensors padded to 128 partitions (zero rows 64..127) so every
        # matmul runs in uniform (128,128) PE tile mode — no mode switches.
        txt_sb = smallp.tile([P, DT], BF16, tag="txt")
        nc.vector.memset(txt_sb[T:P, :], 0.0)
        nc.sync.dma_start(out=txt_sb[0:T, :], in_=txt)
        # mask as a [128,128] stationary: column 0 = mask, rest zero -> M=128
        tmask_sb = smallp.tile([P, P], BF16, tag="tmask")
        nc.vector.memset(tmask_sb, 0.0)
        nc.sync.dma_start(out=tmask_sb[0:T, 0:1], in_=tmask)
        ones_sb = smallp.tile([P, 1], BF16, tag="ones")
        nc.vector.memset(ones_sb, 1.0)
        amask_sb = smallp.tile([P, SC], FP32, tag="amask")
        nc.sync.dma_start(out=amask_sb, in_=amask)
        bq_sb = smallp.tile([P, ECH], FP32, tag="bq")
        nc.sync.dma_start(out=bq_sb, in_=bq)
        bk_sb = smallp.tile([P, ECH], FP32, tag="bk")
        nc.sync.dma_start(out=bk_sb, in_=bk)
        bdq_sb = smallp.tile([P, ECH], FP32, tag="bdq")
        nc.sync.dma_start(out=bdq_sb, in_=bdq)
        bdk_sb = smallp.tile([P, ECH], FP32, tag="bdk")
        nc.sync.dma_start(out=bdk_sb, in_=bdk)
        bvT_sb = smallp.tile([P, ECH], FP32, tag="bvT")
        nc.sync.dma_start(out=bvT_sb, in_=bv.rearrange("(c p) -> p c", p=P))

        wdq_sb = wpool.tile([P, DC, E], BF16, tag="wdq")
        nc.sync.dma_start(out=wdq_sb, in_=wdq)
        wdk_sb = wpool.tile([P, DC, E], BF16, tag="wdk")
        nc.sync.dma_start(out=wdk_sb, in_=wdk)

        # ---- phase A: pooled text + gates ----
        # pr row 0, cols 0:768 = sum_t txt[t,:]*mask[t]; col 768 = sum_t mask[t]
        pr = scp.tile([P, 769], FP32, tag="sc")
        nc.tensor.matmul(pr[:, 0:512], lhsT=tmask_sb, rhs=txt_sb[:, 0:512],
                         start=True, stop=True)
        nc.tensor.matmul(pr[:, 512:768], lhsT=tmask_sb, rhs=txt_sb[:, 512:768],
                         start=True, stop=True)
        nc.tensor.matmul(pr[:, 768:769], lhsT=tmask_sb, rhs=ones_sb,
                         start=True, stop=True)
        rmsum = smallp.tile([1, 1], FP32, tag="rmsum")
        nc.vector.reciprocal(rmsum, pr[0:1, 768:769])
        prow = smallp.tile([1, DT], BF16, tag="prow")
        nc.vector.tensor_scalar(prow, pr[0:1, 0:768], rmsum, None, OP.mult)

        # scatter pooled row -> poolT [128, 6] (dt on partitions) via tiny
        # SBUF->SBUF DMA (dt = c*128 + p)
        poolT = smallp.tile([P, DC], BF16, tag="poolT")
        for c in range(DC):
            nc.sync.dma_start(
                out=poolT[:, c:c + 1],
                in_=prow[0:1, c * P:(c + 1) * P],
            )

        # gates: g = 1 + sigmoid(pool @ Wd + bd); also g*b for fused bias
        gq_sb = smallp.tile([P, ECH], FP32, tag="gq")
        gk_sb = smallp.tile([P, ECH], FP32, tag="gk")
        gbq_sb = smallp.tile([P, ECH], FP32, tag="gbq")
        gbk_sb = smallp.tile([P, ECH], FP32, tag="gbk")
        for (wd_sb, bd_sb, b_sb, g_sb, gb_sb) in (
            (wdq_sb, bdq_sb, bq_sb, gq_sb, gbq_sb),
            (wdk_sb, bdk_sb, bk_sb, gk_sb, gbk_sb),
        ):
            for ec in range(ECH):
                gp = accp.tile([P, 512], FP32, tag="acc")
                for c in range(DC):
                    nc.tensor.matmul(
                        gp[:, 0:1],
                        lhsT=wd_sb[:, c, ec * P:(ec + 1) * P],
                        rhs=poolT[:, c:c + 1],
                        start=(c == 0), stop=(c == DC - 1),
                    )
                # sigmoid(x + bd), then +1 and gate*bias products
                nc.scalar.activation(g_sb[:, ec:ec + 1], gp[:, 0:1], AF.Sigmoid,
                                     bias=bd_sb[:, ec:ec + 1])
            nc.vector.tensor_scalar(g_sb, g_sb, 1.0, None, OP.add)
            nc.vector.tensor_mul(gb_sb, g_sb, b_sb)

        # ---- phase B: big loads ----
        # SWDGE (gpsimd) queue: keeps the multi-MB loads off the HWDGE queue
        # that the gate chain's tiny DMAs (poolT) need early.
        xT_sb = xpool.tile([P, CC, S], BF16, tag="xT")
        nc.gpsimd.dma_start(out=xT_sb, in_=xT)
        wq_sb = wpool.tile([P, CC, E], BF16, tag="wq")
        nc.gpsimd.dma_start(out=wq_sb, in_=wq)
        wk_sb = wpool.tile([P, CC, E], BF16, tag="wk")
        nc.gpsimd.dma_start(out=wk_sb, in_=wk)
        wv_sb = wpool.tile([P, CC, E], BF16, tag="wv")
        nc.gpsimd.dma_start(out=wv_sb, in_=wv)

        # ---- phase C: projections ----
        # V first: it does not depend on the gate chain, so the PE has work
        # while pool/gates resolve. Vaug carries a ones column per head
        # (softmax denominator lands in ctx row 64).
        Vaug = vpool.tile([P, SC, HPC, Dh + 1], BF16, tag="Vaug")
        for t in range(SC):
            ps = accp.tile([P, 512], FP32, tag="acc")
            for c in range(CC):
                nc.tensor.matmul(
                    ps,
                    lhsT=xT_sb[:, c, t * P:(t + 1) * P],
                    rhs=wv_sb[:, c, :],
                    start=(c == 0), stop=(c == CC - 1),
                )
            nc.vector.tensor_copy(
                Vaug[:, t, :, 0:Dh],
                ps.rearrange("p (h d) -> p h d", h=HPC),
            )
            nc.vector.memset(Vaug[:, t, :, Dh:Dh + 1], 1.0)

        # QT packed [e-part, e-chunk, s]; KT per-head zero-padded to full 128
        # partitions (head h real on partitions (h%2)*64.., zeros elsewhere) so
        # score matmuls contract K=128 in the same (128,128) mode as the rest
        # (row-tiled 64x128 scores measured slower: no overlap + mode switches).
        QT = qkpool.tile([P, ECH, S], BF16, tag="QT")
        KTp = qkpool.tile([P, HPC, S], BF16, tag="KTp")
        nc.gpsimd.memset(KTp, 0.0)
        for ec in range(ECH):
            for ss in range(SBL):
                sl = slice(ss * 512, (ss + 1) * 512)
                ps = accp.tile([P, 512], FP32, tag="acc", name="psq")
                for c in range(CC):
                    nc.tensor.matmul(
                        ps,
                        lhsT=wq_sb[:, c, ec * P:(ec + 1) * P],
                        rhs=xT_sb[:, c, sl],
                        start=(c == 0), stop=(c == CC - 1),
                    )
                # (x@W)*g + g*b fused into eviction, cast bf16
                nc.vector.tensor_scalar(
                    QT[:, ec, sl], ps,
                    gq_sb[:, ec:ec + 1], gbq_sb[:, ec:ec + 1],
                    OP.mult, OP.add,
                )
                psk = accp.tile([P, 512], FP32, tag="acc", name="psk")
                for c in range(CC):
                    nc.tensor.matmul(
                        psk,
                        lhsT=wk_sb[:, c, ec * P:(ec + 1) * P],
                        rhs=xT_sb[:, c, sl],
                        start=(c == 0), stop=(c == CC - 1),
                    )
                for hi in range(2):
                    pp = slice(hi * Dh, (hi + 1) * Dh)
                    nc.vector.tensor_scalar(
                        KTp[pp, 2 * ec + hi, sl], psk[pp, :],
                        gk_sb[pp, ec:ec + 1], gbk_sb[pp, ec:ec + 1],
                        OP.mult, OP.add,
                    )

        # ---- phases D+E: attention ----
        # output stays in ctx^T layout [head, d, s]; host transposes to [s, e]
        bisect = os.environ.get("BASS_KERNEL_BISECT", "full")
        if bisect == "c":
            # dummy consumers of QT/KTp/Vaug so DCE keeps phases A-C
            for si in range(SC):
                ot = outp.tile([Dh, S // 4], FP32, tag="outsb", name=f"bo{si}")
                nc.vector.tensor_copy(
                    ot, QT[0:Dh, si % ECH, 0:S // 4])
                nc.vector.tensor_add(
                    ot, ot, KTp[0:Dh, si % HPC, 0:S // 4])
                nc.sync.dma_start(out=out[si % HPC, :, 0:S // 4], in_=ot)
            return
        for sp in range(2):              # s-pair: columns sp*1024 .. +1024
            for h in range(HPC):         # heads sequential, uniform PE mode
                hp, hi = h // 2, h % 2
                ctx_ps = [accp.tile([P, 512], FP32, tag="acc",
                                    name=f"ctx{sp}_{h}_{k}") for k in range(2)]
                for t in range(SC):
                    sps = scp.tile([P, 1024], FP32, tag="sc")
                    for j in range(2):
                        s0 = sp * 1024 + j * 512
                        nc.tensor.matmul(
                            sps[:, j * 512:(j + 1) * 512],
                            lhsT=KTp[:, h, t * P:(t + 1) * P],
                            rhs=QT[:, hp, s0:s0 + 512],
                            start=True, stop=True,
                        )
                    et = etp.tile([P, 1024], BF16, tag="et")
                    nc.scalar.activation(et, sps, AF.Exp,
                                         bias=amask_sb[:, t:t + 1],
                                         scale=0.125)
                    for j in range(2):
                        nc.tensor.matmul(
                            ctx_ps[j][0:Dh + 1, :],
                            lhsT=Vaug[:, t, h, :],
                            rhs=et[:, j * 512:(j + 1) * 512],
                            start=(t == 0), stop=(t == SC - 1),
                        )
                # phase E: normalize in ctx^T layout (no PE transposes):
                # out[d, s] = ctx_unnorm[d, s] * (1/denom[s]) + bv[h*64+d].
                # Evacuate PSUM immediately (one copy) so the bank frees
                # before the slow recip/broadcast chain.
                for j in range(2):
                    sb = sp * 2 + j
                    cs = rbp.tile([Dh + 1, 512], FP32, tag="cs")
                    nc.vector.tensor_copy(cs, ctx_ps[j][0:Dh + 1, :])
                    rc = rcp.tile([1, 512], FP32, tag="rc")
                    nc.vector.reciprocal(rc, cs[Dh:Dh + 1, :])
                    rcb = rbp.tile([Dh, 512], FP32, tag="rcb")
                    # replicate the [1,512] row to 64 partitions: zero-step
                    # middle free dim on the single-partition source
                    rc_bcast = bass.AP(
                        tensor=rc.tensor, offset=rc.offset,
                        ap=[list(rc.ap[0]), [0, Dh]] + [list(d) for d in rc.ap[1:]],
                    )
                    nc.sync.dma_start(out=rcb, in_=rc_bcast)
                    ot = outp.tile([Dh, 512], FP32, tag="outsb")
                    nc.vector.tensor_mul(ot, cs[0:Dh, :], rcb)
                    nc.vector.tensor_scalar(
                        ot, ot,
                        bvT_sb[hi * Dh:(hi + 1) * Dh, hp:hp + 1], None,
                        OP.add,
                    )
                    nc.sync.dma_start(
                        out=out[h, :, sb * 512:(sb + 1) * 512], in_=ot)


def _build():
    key = "nc_" + os.environ.get("BASS_KERNEL_BISECT", "full")
    if key in _CACHE:
        return _CACHE[key]
    nc = bacc.Bacc("TRN2", target_bir_lowering=False, debug=False,
                   enable_asserts=False)
    aps = {}

    def din(name, shape, dt):
        aps[name] = nc.dram_tensor(name, shape, dt, kind="ExternalInput").ap()

    din("xT", [DV, S], BF16)
    din("wq", [DV, E], BF16)
    din("wk", [DV, E], BF16)
    din("wv", [DV, E], BF16)
    din("wdq", [DT, E], BF16)
    din("wdk", [DT, E], BF16)
    din("txt", [T, DT], BF16)
    din("tmask", [T, 1], BF16)
    din("amask", [S], FP32)
    din("bq", [E], FP32)
    din("bk", [E], FP32)
    din("bv", [E], FP32)
    din("bdq", [E], FP32)
    din("bdk", [E], FP32)
    aps["out"] = nc.dram_tensor("out", [HPC, Dh, S], FP32,
                                kind="ExternalOutput").ap()

    with tile.TileContext(nc) as tc:
        _emit(tc, aps)
    nc.compile()
    _CACHE[key] = nc
    return nc


def kernel(**inputs):
    global last_results
    hs = np.asarray(inputs["hidden_states"], dtype=np.float32)
    amask = np.asarray(inputs["attention_mask"], dtype=np.float32)
    txt = np.asarray(inputs["txt_embedding"], dtype=np.float32)
    tmask = np.asarray(inputs["txt_attention_mask"], dtype=np.float32)
    Wq = np.asarray(inputs["Wq"], dtype=np.float32)
    Wk = np.asarray(inputs["Wk"], dtype=np.float32)
    Wv = np.asarray(inputs["Wv"], dtype=np.float32)
    Wdq = np.asarray(inputs["Wdq"], dtype=np.float32)
    Wdk = np.asarray(inputs["Wdk"], dtype=np.float32)
    bq = np.asarray(inputs["bq"], dtype=np.float32)
    bk = np.asarray(inputs["bk"], dtype=np.float32)
    bv = np.asarray(inputs["bv"], dtype=np.float32)
    bdq = np.asarray(inputs["bdq"], dtype=np.float32)
    bdk = np.asarray(inputs["bdk"], dtype=np.float32)

    nc = _build()

    in_maps = []
    for c in range(NCORES):
        b, g = c // 2, c % 2
        cols = slice(g * E, (g + 1) * E)
        in_maps.append({
            "xT": np.ascontiguousarray(hs[b].T).astype(BF16_NP),
            "wq": Wq[:, cols].astype(BF16_NP),
            "wk": Wk[:, cols].astype(BF16_NP),
            "wv": Wv[:, cols].astype(BF16_NP),
            "wdq": Wdq[:, cols].astype(BF16_NP),
            "wdk": Wdk[:, cols].astype(BF16_NP),
            "txt": txt[b].astype(BF16_NP),
            "tmask": tmask[b].astype(BF16_NP),
            "amask": np.ascontiguousarray(amask[b, 0, 0]),
            "bq": np.ascontiguousarray(bq[cols]),
            "bk": np.ascontiguousarray(bk[cols]),
            "bv": np.ascontiguousarray(bv[cols]),
            "bdq": np.ascontiguousarray(bdq[cols]),
            "bdk": np.ascontiguousarray(bdk[cols]),
        })

    tr = int(os.environ.get("BASS_KERNEL_TRACE", "0"))
    if tr == 2:
        # warm the NEFF (compile+load+run untraced), then trace a second run
        run_bass_kernel_spmd(nc, in_maps, list(range(NCORES)), trace=False)
    res = run_bass_kernel_spmd(nc, in_maps, list(range(NCORES)), trace=bool(tr))
    last_results = res

    outp = np.empty((B, S, DV), dtype=np.float32)
    for c in range(NCORES):
        b, g = c // 2, c % 2
        # device output is ctx^T [head, d, s] -> [s, head*64+d]
        co = res.results[c]["out"].transpose(2, 0, 1).reshape(S, E)
        outp[b, :, g * E:(g + 1) * E] = co
    return outp



# revision 4
# speedup vs baseline: 1.2439x; 1.2439x over previous
"""BertImageSelfAttention Trainium2 kernel.

Shapes (fixed): hidden_states [4, 2048, 1024], 16 heads x 64, text [4, 64, 768].
Sharding: 8 cores = 4 batches x 2 head-groups (8 heads each). Each core computes
its batch's attention context for its 8 heads; host reassembles [4, 2048, 1024].

Per-core device pipeline (all matmuls bf16 with fp32 PSUM accumulation). The
scalar engine's Exp over the full S x S scores (33.5M elem/core at 1 elem/
cycle/lane) is the pacing engine, so the kernel is organized to saturate it:

  A. pooled text -> dynamic Q/K gates (tiny matmuls + sigmoid).
  B. loads: gate-critical tensors on the HWDGE queue first; x^T split into
     four s-slices on the SWDGE queue so projections start mid-load.
  C. Q/K projections for head-pair 0 run first; attention head 0 starts as
     soon as its first K/Q blocks land (~15us in). The V projection and the
     remaining Q/K e-chunks are dosed into the tensor engine's slack inside
     later heads' attention loops, underneath the Exp-bound steady state.
  D. per head: S^T tiles = K^T.T @ Q^T (keys on partitions), ACT Exp with
     scale=1/8 and per-partition bias=attention_mask, ctx^T[65,512] +=
     Vaug.T @ E^T over 16 key chunks (row 64 = softmax denominator).
  E. per (s-half, head): evict ctx^T to SBUF, DMA-gather the [1,1024]
     denominator row across 128 partitions, one cheap [128,8] reciprocal,
     DMA scatter+broadcast back to [64,1024], fused multiply + bias add,
     DMA out [64,1024] fp32 (output stays ctx^T; host transposes).
"""

import os

import numpy as np
import ml_dtypes

import concourse.bass as bass
import concourse.bacc as bacc
import concourse.tile as tile
from concourse import mybir
from concourse.bass_utils import run_bass_kernel_spmd

P = 128
B, S, DV = 4, 2048, 1024
H, Dh = 16, 64
T, DT = 64, 768
NCORES = 8
E = 512          # head-group width (8 heads x 64)
CC = DV // P     # 8 contraction chunks for projections
ECH = E // P     # 4 e-chunks
DC = DT // P     # 6 text-dim chunks
SC = S // P      # 16 seq chunks of 128
SBL = S // 512   # 4 seq blocks of 512
HPC = 8          # heads per core

FP32 = mybir.dt.float32
BF16 = mybir.dt.bfloat16
AF = mybir.ActivationFunctionType
OP = mybir.AluOpType

BF16_NP = ml_dtypes.bfloat16

_CACHE = {}

# module-level stash of the last BassKernelResults (for test.py introspection)
last_results = None


def _emit(tc, aps):
    nc = tc.nc
    xT = aps["xT"].rearrange("(c p) s -> p c s", p=P)          # [128, 8, 2048]
    wq = aps["wq"].rearrange("(c p) e -> p c e", p=P)          # [128, 8, 512]
    wk = aps["wk"].rearrange("(c p) e -> p c e", p=P)
    wv = aps["wv"].rearrange("(c p) e -> p c e", p=P)
    wdq = aps["wdq"].rearrange("(c p) e -> p c e", p=P)        # [128, 6, 512]
    wdk = aps["wdk"].rearrange("(c p) e -> p c e", p=P)
    txt = aps["txt"]                                           # [64, 768] bf16
    tmask = aps["tmask"]                                       # [64, 1] bf16
    amask = aps["amask"].rearrange("(c p) -> p c", p=P)        # [128, 16]
    bq = aps["bq"].rearrange("(c p) -> p c", p=P)              # [128, 4]
    bk = aps["bk"].rearrange("(c p) -> p c", p=P)
    bdq = aps["bdq"].rearrange("(c p) -> p c", p=P)
    bdk = aps["bdk"].rearrange("(c p) -> p c", p=P)
    bv = aps["bv"]                                             # [512]
    out = aps["out"]                                           # [8, 64, 2048] f32

    from contextlib import ExitStack

    with ExitStack() as ctx:
        wpool = ctx.enter_context(tc.tile_pool(name="wpool", bufs=1))
        xpool = ctx.enter_context(tc.tile_pool(name="xpool", bufs=1))
        qkpool = ctx.enter_context(tc.tile_pool(name="qkpool", bufs=1))
        vpool = ctx.enter_context(tc.tile_pool(name="vpool", bufs=1))
        etp = ctx.enter_context(tc.tile_pool(name="etp", bufs=4))
        rbp = ctx.enter_context(tc.tile_pool(name="rbp", bufs=2))
        outp = ctx.enter_context(tc.tile_pool(name="outp", bufs=2))
        smallp = ctx.enter_context(tc.tile_pool(name="smallp", bufs=1))
        rcp = ctx.enter_context(tc.tile_pool(name="rcp", bufs=2))
        # PSUM: 8 banks = scp 2 x [128,1024] (2 banks each) + accp 2 x 1 bank
        # (ctx accumulators) + pjp 2 x 1 bank (projection scratch, dosed into
        # the attention loop — separate pool so rotation can't land on a
        # live ctx accumulator).
        accp = ctx.enter_context(tc.tile_pool(name="accp", bufs=2, space="PSUM"))
        pjp = ctx.enter_context(tc.tile_pool(name="pjp", bufs=2, space="PSUM"))
        scp = ctx.enter_context(tc.tile_pool(name="scp", bufs=2, space="PSUM"))

        # ---- gate-critical small loads (HWDGE queue) ----
        # text tensors padded to 128 partitions (zero rows 64..127) so every
        # matmul runs in uniform (128,128) PE tile mode — no mode switches.
        txt_sb = smallp.tile([P, DT], BF16, tag="txt")
        nc.vector.memset(txt_sb[T:P, :], 0.0)
        nc.sync.dma_start(out=txt_sb[0:T, :], in_=txt)
        # mask as a [128,128] stationary: column 0 = mask, rest zero -> M=128
        tmask_sb = smallp.tile([P, P], BF16, tag="tmask")
        nc.vector.memset(tmask_sb, 0.0)
        nc.sync.dma_start(out=tmask_sb[0:T, 0:1], in_=tmask)
        ones_sb = smallp.tile([P, 1], BF16, tag="ones")
        nc.vector.memset(ones_sb, 1.0)
        amask_sb = smallp.tile([P, SC], FP32, tag="amask")
        nc.sync.dma_start(out=amask_sb, in_=amask)
        bq_sb = smallp.tile([P, ECH], FP32, tag="bq")
        nc.sync.dma_start(out=bq_sb, in_=bq)
        bk_sb = smallp.tile([P, ECH], FP32, tag="bk")
        nc.sync.dma_start(out=bk_sb, in_=bk)
        bdq_sb = smallp.tile([P, ECH], FP32, tag="bdq")
        nc.sync.dma_start(out=bdq_sb, in_=bdq)
        bdk_sb = smallp.tile([P, ECH], FP32, tag="bdk")
        nc.sync.dma_start(out=bdk_sb, in_=bdk)
        bvT_sb = smallp.tile([P, ECH], FP32, tag="bvT")
        nc.sync.dma_start(out=bvT_sb, in_=bv.rearrange("(c p) -> p c", p=P))

        wdq_sb = wpool.tile([P, DC, E], BF16, tag="wdq")
        nc.sync.dma_start(out=wdq_sb, in_=wdq)
        wdk_sb = wpool.tile([P, DC, E], BF16, tag="wdk")
        nc.sync.dma_start(out=wdk_sb, in_=wdk)

        # ---- big loads (SWDGE/gpsimd queue) ----
        # Q/K weights first (head-pair 0's projections are start-critical),
        # then xT in four s-slices so the ss=0 projections run mid-load.
        wq_sb = wpool.tile([P, CC, E], BF16, tag="wq")
        nc.gpsimd.dma_start(out=wq_sb, in_=wq)
        wk_sb = wpool.tile([P, CC, E], BF16, tag="wk")
        nc.gpsimd.dma_start(out=wk_sb, in_=wk)
        xT_sb = xpool.tile([P, CC, S], BF16, tag="xT")
        for ss in range(SBL):
            sl = slice(ss * 512, (ss + 1) * 512)
            nc.gpsimd.dma_start(out=xT_sb[:, :, sl], in_=xT[:, :, sl])
        wv_sb = wpool.tile([P, CC, E], BF16, tag="wv")
        nc.gpsimd.dma_start(out=wv_sb, in_=wv)

        # ---- phase A: pooled text + gates ----
        # pr row 0, cols 0:768 = sum_t txt[t,:]*mask[t]; col 768 = sum_t mask[t]
        pr = scp.tile([P, 1024], FP32, tag="sc")
        nc.tensor.matmul(pr[:, 0:512], lhsT=tmask_sb, rhs=txt_sb[:, 0:512],
                         start=True, stop=True)
        nc.tensor.matmul(pr[:, 512:768], lhsT=tmask_sb, rhs=txt_sb[:, 512:768],
                         start=True, stop=True)
        nc.tensor.matmul(pr[:, 768:769], lhsT=tmask_sb, rhs=ones_sb,
                         start=True, stop=True)
        rmsum = smallp.tile([1, 1], FP32, tag="rmsum")
        nc.vector.reciprocal(rmsum, pr[0:1, 768:769])
        prow = smallp.tile([1, DT], BF16, tag="prow")
        nc.vector.tensor_scalar(prow, pr[0:1, 0:768], rmsum, None, OP.mult)

        # scatter pooled row -> poolT [128, 6] (dt on partitions) via tiny
        # SBUF->SBUF DMA (dt = c*128 + p)
        poolT = smallp.tile([P, DC], BF16, tag="poolT")
        for c in range(DC):
            nc.sync.dma_start(
                out=poolT[:, c:c + 1],
                in_=prow[0:1, c * P:(c + 1) * P],
            )

        # gates: g = 1 + sigmoid(pool @ Wd + bd); also g*b for fused bias
        gq_sb = smallp.tile([P, ECH], FP32, tag="gq")
        gk_sb = smallp.tile([P, ECH], FP32, tag="gk")
        gbq_sb = smallp.tile([P, ECH], FP32, tag="gbq")
        gbk_sb = smallp.tile([P, ECH], FP32, tag="gbk")
        for (wd_sb, bd_sb, b_sb, g_sb, gb_sb) in (
            (wdq_sb, bdq_sb, bq_sb, gq_sb, gbq_sb),
            (wdk_sb, bdk_sb, bk_sb, gk_sb, gbk_sb),
        ):
            for ec in range(ECH):
                gp = pjp.tile([P, 512], FP32, tag="acc")
                for c in range(DC):
                    nc.tensor.matmul(
                        gp[:, 0:1],
                        lhsT=wd_sb[:, c, ec * P:(ec + 1) * P],
                        rhs=poolT[:, c:c + 1],
                        start=(c == 0), stop=(c == DC - 1),
                    )
                # sigmoid(x + bd), then +1 and gate*bias products
                nc.scalar.activation(g_sb[:, ec:ec + 1], gp[:, 0:1], AF.Sigmoid,
                                     bias=bd_sb[:, ec:ec + 1])
            nc.vector.tensor_scalar(g_sb, g_sb, 1.0, None, OP.add)
            nc.vector.tensor_mul(gb_sb, g_sb, b_sb)

        # ---- projection emitters (dosed into the attention loop) ----
        QT = qkpool.tile([P, ECH, S], BF16, tag="QT")
        KTp = qkpool.tile([P, HPC, S], BF16, tag="KTp")
        nc.gpsimd.memset(KTp, 0.0)
        Vaug = vpool.tile([P, SC, HPC, Dh + 1], BF16, tag="Vaug")

        def emit_qk_half(ec, ss, which):
            """One projection half-group: 8 matmuls + eviction for Q or K."""
            sl = slice(ss * 512, (ss + 1) * 512)
            if which == "q":
                ps = pjp.tile([P, 512], FP32, tag="acc", name=f"psq{ec}_{ss}")
                for c in range(CC):
                    nc.tensor.matmul(
                        ps,
                        lhsT=wq_sb[:, c, ec * P:(ec + 1) * P],
                        rhs=xT_sb[:, c, sl],
                        start=(c == 0), stop=(c == CC - 1),
                    )
                # (x@W)*g + g*b fused into eviction, cast bf16
                nc.vector.tensor_scalar(
                    QT[:, ec, sl], ps,
                    gq_sb[:, ec:ec + 1], gbq_sb[:, ec:ec + 1],
                    OP.mult, OP.add,
                )
            else:
                psk = pjp.tile([P, 512], FP32, tag="acc", name=f"psk{ec}_{ss}")
                for c in range(CC):
                    nc.tensor.matmul(
                        psk,
                        lhsT=wk_sb[:, c, ec * P:(ec + 1) * P],
                        rhs=xT_sb[:, c, sl],
                        start=(c == 0), stop=(c == CC - 1),
                    )
                # K^T per-head zero-padded to 128 partitions (head h real on
                # partitions (h%2)*64..) so score matmuls contract K=128 in
                # the same (128,128) mode as everything else.
                for hi in range(2):
                    pp = slice(hi * Dh, (hi + 1) * Dh)
                    nc.vector.tensor_scalar(
                        KTp[pp, 2 * ec + hi, sl], psk[pp, :],
                        gk_sb[pp, ec:ec + 1], gbk_sb[pp, ec:ec + 1],
                        OP.mult, OP.add,
                    )

        def emit_v(t):
            """V projection t-chunk: 8 matmuls + eviction into Vaug."""
            ps = pjp.tile([P, 512], FP32, tag="acc", name=f"psv{t}")
            for c in range(CC):
                nc.tensor.matmul(
                    ps,
                    lhsT=xT_sb[:, c, t * P:(t + 1) * P],
                    rhs=wv_sb[:, c, :],
                    start=(c == 0), stop=(c == CC - 1),
                )
            nc.vector.tensor_copy(
                Vaug[:, t, :, 0:Dh],
                ps.rearrange("p (h d) -> p h d", h=HPC),
            )
            nc.vector.memset(Vaug[:, t, :, Dh:Dh + 1], 1.0)

        # head-pair 0's projections run up front (start-critical).
        for ss in range(SBL):
            emit_qk_half(0, ss, "q")
            emit_qk_half(0, ss, "k")

        # Remaining projections are dosed into attention t-iterations:
        # dose[(h, sp, t)] = list of emitters to run before that tile's
        # score matmuls. h0/sp0 hosts V (ctx t needs Vaug[t] same iter);
        # h1, h3, h5 host the ec1..ec3 Q/K half-groups (one per 4 iters).
        dose = {}
        for t in range(SC):
            dose[(0, 0, t)] = [lambda t=t: emit_v(t)]
        for ec in (1, 2, 3):
            h = 2 * ec - 1
            k = 0
            for ss in range(SBL):
                for which in ("q", "k"):
                    sp, t = divmod(4 * k, SC)
                    dose.setdefault((h, sp, t), []).append(
                        lambda ec=ec, ss=ss, w=which: emit_qk_half(ec, ss, w))
                    k += 1

        # ---- phases D+E: attention ----
        # output stays in ctx^T layout [head, d, s]; host transposes to [s, e]
        for h in range(HPC):             # heads sequential, uniform PE mode
            hp, hi = h // 2, h % 2
            for sp in range(2):          # s-half: columns sp*1024 .. +1024
                ctx_ps = [accp.tile([P, 512], FP32, tag="acc",
                                    name=f"ctx{sp}_{h}_{k}") for k in range(2)]
                for t in range(SC):
                    for fn in dose.get((h, sp, t), ()):
                        fn()
                    sps = scp.tile([P, 1024], FP32, tag="sc")
                    for j in range(2):
                        s0 = sp * 1024 + j * 512
                        nc.tensor.matmul(
                            sps[:, j * 512:(j + 1) * 512],
                            lhsT=KTp[:, h, t * P:(t + 1) * P],
                            rhs=QT[:, hp, s0:s0 + 512],
                            start=True, stop=True,
                        )
                    et = etp.tile([P, 1024], BF16, tag="et")
                    nc.scalar.activation(et, sps, AF.Exp,
                                         bias=amask_sb[:, t:t + 1],
                                         scale=0.125)
                    for j in range(2):
                        nc.tensor.matmul(
                            ctx_ps[j][0:Dh + 1, :],
                            lhsT=Vaug[:, t, h, :],
                            rhs=et[:, j * 512:(j + 1) * 512],
                            start=(t == 0), stop=(t == SC - 1),
                        )
                # phase E: normalize in ctx^T layout (no PE transposes):
                # out[d, s] = ctx_unnorm[d, s] * (1/denom[s]) + bv[h*64+d].
                # Evacuate PSUM immediately so the banks free early.
                cs = rbp.tile([Dh + 1, 1024], FP32, tag="cs")
                for j in range(2):
                    nc.vector.tensor_copy(
                        cs[:, j * 512:(j + 1) * 512], ctx_ps[j][0:Dh + 1, :])
                # denominator row [1,1024] -> [128,8] across partitions, one
                # cheap reciprocal, then scatter back + partition-broadcast.
                rpack = rcp.tile([P, 8], FP32, tag="rpack")
                nc.sync.dma_start(out=rpack, in_=cs[Dh:Dh + 1, :])
                nc.vector.reciprocal(rpack, rpack)
                rrow = rcp.tile([1, 1024], FP32, tag="rrow")
                nc.sync.dma_start(out=rrow, in_=rpack)
                rcb = rcp.tile([Dh, 1024], FP32, tag="rcb")
                rr_bcast = bass.AP(
                    tensor=rrow.tensor, offset=rrow.offset,
                    ap=[list(rrow.ap[0]), [0, Dh]] + [list(d) for d in rrow.ap[1:]],
                )
                nc.sync.dma_start(out=rcb, in_=rr_bcast)
                ot = outp.tile([Dh, 1024], FP32, tag="outsb")
                nc.vector.tensor_mul(ot, cs[0:Dh, :], rcb)
                nc.vector.tensor_scalar(
                    ot, ot,
                    bvT_sb[hi * Dh:(hi + 1) * Dh, hp:hp + 1], None,
                    OP.add,
                )
                nc.sync.dma_start(
                    out=out[h, :, sp * 1024:(sp + 1) * 1024], in_=ot)


def _build():
    key = "nc"
    if key in _CACHE:
        return _CACHE[key]
    nc = bacc.Bacc("TRN2", target_bir_lowering=False, debug=False,
                   enable_asserts=False)
    aps = {}

    def din(name, shape, dt):
        aps[name] = nc.dram_tensor(name, shape, dt, kind="ExternalInput").ap()

    din("xT", [DV, S], BF16)
    din("wq", [DV, E], BF16)
    din("wk", [DV, E], BF16)
    din("wv", [DV, E], BF16)
    din("wdq", [DT, E], BF16)
    din("wdk", [DT, E], BF16)
    din("txt", [T, DT], BF16)
    din("tmask", [T, 1], BF16)
    din("amask", [S], FP32)
    din("bq", [E], FP32)
    din("bk", [E], FP32)
    din("bv", [E], FP32)
    din("bdq", [E], FP32)
    din("bdk", [E], FP32)
    aps["out"] = nc.dram_tensor("out", [HPC, Dh, S], FP32,
                                kind="ExternalOutput").ap()

    with tile.TileContext(nc) as tc:
        _emit(tc, aps)
    nc.compile()
    _CACHE[key] = nc
    return nc


def kernel(**inputs):
    global last_results
    hs = np.asarray(inputs["hidden_states"], dtype=np.float32)
    amask = np.asarray(inputs["attention_mask"], dtype=np.float32)
    txt = np.asarray(inputs["txt_embedding"], dtype=np.float32)
    tmask = np.asarray(inputs["txt_attention_mask"], dtype=np.float32)
    Wq = np.asarray(inputs["Wq"], dtype=np.float32)
    Wk = np.asarray(inputs["Wk"], dtype=np.float32)
    Wv = np.asarray(inputs["Wv"], dtype=np.float32)
    Wdq = np.asarray(inputs["Wdq"], dtype=np.float32)
    Wdk = np.asarray(inputs["Wdk"], dtype=np.float32)
    bq = np.asarray(inputs["bq"], dtype=np.float32)
    bk = np.asarray(inputs["bk"], dtype=np.float32)
    bv = np.asarray(inputs["bv"], dtype=np.float32)
    bdq = np.asarray(inputs["bdq"], dtype=np.float32)
    bdk = np.asarray(inputs["bdk"], dtype=np.float32)

    nc = _build()

    in_maps = []
    for c in range(NCORES):
        b, g = c // 2, c % 2
        cols = slice(g * E, (g + 1) * E)
        in_maps.append({
            "xT": np.ascontiguousarray(hs[b].T).astype(BF16_NP),
            "wq": Wq[:, cols].astype(BF16_NP),
            "wk": Wk[:, cols].astype(BF16_NP),
            "wv": Wv[:, cols].astype(BF16_NP),
            "wdq": Wdq[:, cols].astype(BF16_NP),
            "wdk": Wdk[:, cols].astype(BF16_NP),
            "txt": txt[b].astype(BF16_NP),
            "tmask": tmask[b].astype(BF16_NP),
            "amask": np.ascontiguousarray(amask[b, 0, 0]),
            "bq": np.ascontiguousarray(bq[cols]),
            "bk": np.ascontiguousarray(bk[cols]),
            "bv": np.ascontiguousarray(bv[cols]),
            "bdq": np.ascontiguousarray(bdq[cols]),
            "bdk": np.ascontiguousarray(bdk[cols]),
        })

    tr = int(os.environ.get("BASS_KERNEL_TRACE", "0"))
    if tr == 2:
        # warm the NEFF (compile+load+run untraced), then trace a second run
        run_bass_kernel_spmd(nc, in_maps, list(range(NCORES)), trace=False)
    res = run_bass_kernel_spmd(nc, in_maps, list(range(NCORES)), trace=bool(tr))
    last_results = res

    outp = np.empty((B, S, DV), dtype=np.float32)
    for c in range(NCORES):
        b, g = c // 2, c % 2
        # device output is ctx^T [head, d, s] -> [s, head*64+d]
        co = res.results[c]["out"].transpose(2, 0, 1).reshape(S, E)
        outp[b, :, g * E:(g + 1) * E] = co
    return outp
